# revision 55
# baseline (speedup 1.0000x reference)
"""Adversarial-embedding kernel for Trainium2 (8 NeuronCores, SPMD).

Computes (per the nn_Adversarial reference):
    L[t]    = gumbel-argmax over non-pad positions of dpadder[t, :]  (key 42)
    A       = emb, with row (t, L[t]) scaled by (1 + EPS / ||emb[t, L[t]]||)
Returns (A, L).

Sharding: data-parallel over tlen across 8 cores. Each core gets a
(256, 32, 1024) f32 shard flattened to (8192, 1024) plus a (256, 1) int32
row-index tensor. On device: a chunked bulk DRAM->DRAM copy (the
memory-bound part, ~64MiB of HBM traffic per core), an indirect-DMA gather
of the 256 selected rows, norm+scale on DVE, and indirect-DMA scatters of
the scaled rows into the output, ordered after the copy.
"""

import functools
import os

import numpy as np

TLEN, BZ, EMB = 2048, 32, 1024
NCORES = 8
SHARD_T = TLEN // NCORES  # 256
ROWS = SHARD_T * BZ  # 8192
EPSILON = 0.05


def _compute_L(dpadder: np.ndarray) -> np.ndarray:
    """Mirror of reference._sample_valid_indices, on host CPU."""
    import jax
    import jax.numpy as jnp

    cpu = jax.devices("cpu")[0]
    with jax.default_device(cpu):
        g = jax.random.gumbel(jax.random.key(42), (TLEN, BZ), dtype=jnp.float32)
        dp = jnp.asarray(dpadder)
        scores = jnp.where(dp != 1, g, -jnp.inf)
        L = jnp.argmax(scores, axis=1)
    return np.asarray(L)


@functools.lru_cache(maxsize=1)
def _build_program(n_copy_chunks: int = 16):
    from concourse import bacc, bass, mybir
    from concourse.tile import TileContext
    from concourse.tile_rust import add_dep_helper

    f32 = mybir.dt.float32
    i32 = mybir.dt.int32

    # Bacc (not plain Bass): its finalize() runs generate_event_semaphores,
    # which legalizes instructions with >1 sync wait (TRN2 allows only one
    # wait per instruction) by splitting through event semaphores.
    nc = bacc.Bacc()
    emb = nc.declare_dram_parameter("emb", [ROWS, EMB], f32, isOutput=False)
    # idx col 0: global row (for the gather from emb); col 1: row local to
    # the half-output tensor (for the scatter).
    idx = nc.declare_dram_parameter("idx", [SHARD_T, 2], i32, isOutput=False)
    # Two disjoint output tensors (row halves): the indirect scatters then
    # target separate tensors, so Tile's conservative whole-tensor WAW
    # tracking can neither serialize the two scatters against each other
    # nor against the other half's copy chunks — the half-0 scatter fires
    # at copy half-time and only the half-1 scatter remains on the tail.
    outs = [
        nc.declare_dram_parameter(f"out{h}", [ROWS // 2, EMB], f32, isOutput=True)
        for h in range(2)
    ]

    def raw(inst):
        return getattr(inst, "ins", inst)

    with TileContext(nc) as tc:
        with tc.tile_pool(name="sbuf", bufs=1) as pool:
            # Selected-row indices: cols 0-1 = global rows (halves 0/1) for
            # the gathers, cols 2-3 = half-local rows for the scatters.
            idx_t = pool.tile([128, 4], i32)
            nc.sync.dma_start(out=idx_t[:, 0:1], in_=idx[0:128, 0:1])
            nc.sync.dma_start(out=idx_t[:, 1:2], in_=idx[128:256, 0:1])
            nc.sync.dma_start(out=idx_t[:, 2:3], in_=idx[0:128, 1:2])
            nc.sync.dma_start(out=idx_t[:, 3:4], in_=idx[128:256, 1:2])

            # Bulk copy: out[:] = emb[:], straight DRAM->DRAM (most
            # engine-efficient path: each byte passes one SDMA engine once).
            # Split into chunks (finer descriptors -> all 16 SDMA engines
            # saturate early) and alternate the two HWDGE rings (SP + ACT).
            # Half-0: fine chunks for a fast ramp. Half-1 (whose completion
            # gates the tail scatter): fewer, larger chunks so the scatter
            # waits on fewer DMA-lane completion receipts.
            half_rows = ROWS // 2
            chunks_per_half = (n_copy_chunks // 2, 4)
            copy_halves = ([], [])
            k = 0
            for h in range(2):
                n_ch = chunks_per_half[h]
                assert half_rows % n_ch == 0
                rp = half_rows // n_ch
                for ch in range(n_ch):
                    dsl = slice(ch * rp, (ch + 1) * rp)
                    ssl = slice(h * half_rows + ch * rp, h * half_rows + (ch + 1) * rp)
                    eng = nc.sync if k % 2 == 0 else nc.scalar
                    k += 1
                    copy_halves[h].append(
                        eng.dma_start(out=outs[h][dsl, :], in_=emb[ssl, :])
                    )

            # Gather the 2x128 selected rows into one [128, 2, EMB] tile
            # (row h*128+p lands at partition p, slot h).
            rows3 = pool.tile([128, 2, EMB], f32)
            for half in range(2):
                hidx = idx_t[:, half : half + 1]
                rows = rows3[:, half, :]
                nc.gpsimd.indirect_dma_start(
                    out=rows,
                    out_offset=None,
                    in_=emb[:, :],
                    in_offset=bass.IndirectOffsetOnAxis(ap=hidx, axis=0),
                )
                # All compute on DVE (using ACT would pull in LoadActFuncSet
                # table loads that gate the ACT-ring copy chunks, and keeps
                # every op's cross-engine waits trivial): square + reduce for
                # sumsq, then rsqrt via the int32 bit-hack seed + 3 Newton
                # iterations (fp32-accurate), then scale = 1 + eps*rsqrt.
                sq = pool.tile([128, EMB], f32, tag=f"sq{half}")
                ss = pool.tile([128, 1], f32, tag=f"ss{half}")
                nc.vector.tensor_tensor(
                    out=sq[:], in0=rows, in1=rows, op=mybir.AluOpType.mult
                )
                nc.vector.reduce_sum(
                    out=ss[:], in_=sq[:], axis=mybir.AxisListType.X
                )
                # z0 = bitcast_f32(0x5f3759df - (bitcast_i32(ss) >> 1))
                zi = pool.tile([128, 1], i32, tag=f"zi{half}")
                nc.vector.tensor_scalar(
                    out=zi[:],
                    in0=ss[:].bitcast(i32),
                    scalar1=1,
                    scalar2=None,
                    op0=mybir.AluOpType.arith_shift_right,
                )
                # zi = ~(zi) + (0x5f3759df + 1)  ==  0x5f3759df - zi
                # (bitwise and arith ops can't share one TensorScalar)
                nc.vector.tensor_scalar(
                    out=zi[:],
                    in0=zi[:],
                    scalar1=-1,
                    scalar2=None,
                    op0=mybir.AluOpType.bitwise_xor,
                )
                nc.vector.tensor_scalar(
                    out=zi[:],
                    in0=zi[:],
                    scalar1=0x5F3759DF + 1,
                    scalar2=None,
                    op0=mybir.AluOpType.add,
                )
                z = zi[:].bitcast(f32)
                t0 = pool.tile([128, 1], f32, tag=f"t0{half}")
                for _ in range(3):
                    # z <- z * (1.5 - 0.5 * ss * z * z)
                    nc.vector.tensor_tensor(
                        out=t0[:], in0=z, in1=z, op=mybir.AluOpType.mult
                    )
                    nc.vector.tensor_tensor(
                        out=t0[:], in0=t0[:], in1=ss[:], op=mybir.AluOpType.mult
                    )
                    nc.vector.tensor_scalar(
                        out=t0[:],
                        in0=t0[:],
                        scalar1=-0.5,
                        scalar2=1.5,
                        op0=mybir.AluOpType.mult,
                        op1=mybir.AluOpType.add,
                    )
                    nc.vector.tensor_tensor(
                        out=zi[:].bitcast(f32), in0=z, in1=t0[:],
                        op=mybir.AluOpType.mult,
                    )
                # scale = 1 + EPSILON * rsqrt(sumsq)
                scl = pool.tile([128, 1], f32, tag=f"scl{half}")
                nc.vector.tensor_scalar(
                    out=scl[:],
                    in0=z,
                    scalar1=EPSILON,
                    scalar2=1.0,
                    op0=mybir.AluOpType.mult,
                    op1=mybir.AluOpType.add,
                )
                nc.vector.tensor_scalar_mul(out=rows, in0=rows, scalar1=scl[:, :1])

            # Each scatter lands after ITS half's copy chunks only (disjoint
            # output tensors -> no cross-half or scatter-scatter edges).
            for half in range(2):
                lidx = idx_t[:, 2 + half : 3 + half]
                si = nc.gpsimd.indirect_dma_start(
                    out=outs[half][:, :],
                    out_offset=bass.IndirectOffsetOnAxis(ap=lidx, axis=0),
                    in_=rows3[:, half, :],
                    in_offset=None,
                )
                for ci in copy_halves[half]:
                    add_dep_helper(raw(si), raw(ci), reason="scatter after half copy")

    # run_bass_via_pjrt binds the exec primitive without finalizing; Bacc
    # needs finalize() to run its compile() pipeline (register allocation,
    # event-semaphore legalization of multi-wait instructions, ...).
    nc.finalize()
    return nc


def _install_ntff_hook_shim():
    """This image lacks antenv.axon_hooks (which bass_utils imports for
    trace=True under axon); synthesize it from trn_boot's ctypes NTFF
    profiler. Profiling-only — never needed for plain execution."""
    import sys
    import types

    try:
        import antenv.axon_hooks  # noqa: F401

        return
    except ImportError:
        pass
    try:
        import antenv
        from trn_agent_boot.trn_boot import _ntff_profile_via_ctypes
    except ImportError:
        return
    hook = _ntff_profile_via_ctypes("/opt/axon/libaxon_pjrt.so")
    mod = types.ModuleType("antenv.axon_hooks")
    mod.get_axon_ntff_profile_hook = lambda: hook
    mod.set_axon_ntff_profile_hook = lambda h: None
    sys.modules["antenv.axon_hooks"] = mod
    antenv.axon_hooks = mod


LAST_EXEC_NS = None


def kernel(emb, data, dpadder):
    global LAST_EXEC_NS
    emb = np.ascontiguousarray(np.asarray(emb), dtype=np.float32)
    dpad = np.asarray(dpadder)
    L = _compute_L(dpad)

    from concourse.bass_utils import run_bass_kernel_spmd

    nc = _build_program()

    t = np.arange(SHARD_T, dtype=np.int64)
    in_maps = []
    for s in range(NCORES):
        Ls = L[s * SHARD_T : (s + 1) * SHARD_T].astype(np.int64)
        gidx = (t * BZ + Ls).astype(np.int32)
        lidx = gidx - np.where(t >= SHARD_T // 2, ROWS // 2, 0).astype(np.int32)
        rowidx = np.ascontiguousarray(np.stack([gidx, lidx], axis=1))
        shard = emb[s * SHARD_T : (s + 1) * SHARD_T].reshape(ROWS, EMB)
        in_maps.append({"emb": shard, "idx": rowidx})

    trace = os.environ.get("KERNEL_TRACE", "0") == "1"
    if trace:
        _install_ntff_hook_shim()
    res = run_bass_kernel_spmd(
        nc, in_maps, list(range(NCORES)), trace=trace
    )
    if trace:
        LAST_EXEC_NS = res.exec_time_ns
        print(f"HW exec time: {res.exec_time_ns} ns")
        print(f"mean exec time: {res.mean_exec_time_ns} ns")
        if res.instructions_and_trace is not None:
            print(f"trace: {res.instructions_and_trace[1]}")

    A = np.concatenate(
        [
            np.concatenate([r["out0"], r["out1"]], axis=0).reshape(
                SHARD_T, BZ, EMB
            )
            for r in res.results
        ],
        axis=0,
    )
    L_dtype = np.int64 if np.asarray(data).dtype == np.int64 else np.int32
    return A, L.astype(L_dtype)


# revision 56
# speedup vs baseline: 1.1685x; 1.1685x over previous
"""Adversarial-embedding kernel for Trainium2 (8 NeuronCores, SPMD).

Computes (per the nn_Adversarial reference):
    L[t]    = gumbel-argmax over non-pad positions of dpadder[t, :]  (key 42)
    A       = emb, with row (t, L[t]) scaled by (1 + EPS / ||emb[t, L[t]]||)
Returns (A, L).

Sharding: data-parallel over tlen across 8 cores. Each core gets a
(256, 32, 1024) f32 shard flattened to (8192, 1024) plus a (256, 1) int32
row-index tensor. On device: a chunked bulk DRAM->DRAM copy (the
memory-bound part, ~64MiB of HBM traffic per core), an indirect-DMA gather
of the 256 selected rows, norm+scale on DVE, and indirect-DMA scatters of
the scaled rows into the output, ordered after the copy.
"""

import functools
import os

import numpy as np

TLEN, BZ, EMB = 2048, 32, 1024
NCORES = 8
SHARD_T = TLEN // NCORES  # 256
ROWS = SHARD_T * BZ  # 8192
EPSILON = 0.05


def _compute_L(dpadder: np.ndarray) -> np.ndarray:
    """Mirror of reference._sample_valid_indices, on host CPU."""
    import jax
    import jax.numpy as jnp

    cpu = jax.devices("cpu")[0]
    with jax.default_device(cpu):
        g = jax.random.gumbel(jax.random.key(42), (TLEN, BZ), dtype=jnp.float32)
        dp = jnp.asarray(dpadder)
        scores = jnp.where(dp != 1, g, -jnp.inf)
        L = jnp.argmax(scores, axis=1)
    return np.asarray(L)


@functools.lru_cache(maxsize=1)
def _build_program(n_copy_chunks: int = 16):
    from concourse import bacc, bass, mybir
    from concourse.tile import TileContext
    from concourse.tile_rust import add_dep_helper

    f32 = mybir.dt.float32
    i32 = mybir.dt.int32

    # Bacc (not plain Bass): its finalize() runs generate_event_semaphores,
    # which legalizes instructions with >1 sync wait (TRN2 allows only one
    # wait per instruction) by splitting through event semaphores.
    nc = bacc.Bacc()
    emb = nc.declare_dram_parameter("emb", [ROWS, EMB], f32, isOutput=False)
    # idx col 0: global row (for the gather from emb); col 1: row local to
    # the half-output tensor (for the scatter).
    idx = nc.declare_dram_parameter("idx", [SHARD_T, 2], i32, isOutput=False)
    # Two disjoint output tensors (row halves): the indirect scatters then
    # target separate tensors, so Tile's conservative whole-tensor WAW
    # tracking can neither serialize the two scatters against each other
    # nor against the other half's copy chunks — the half-0 scatter fires
    # at copy half-time and only the half-1 scatter remains on the tail.
    outs = [
        nc.declare_dram_parameter(f"out{h}", [ROWS // 2, EMB], f32, isOutput=True)
        for h in range(2)
    ]

    def raw(inst):
        return getattr(inst, "ins", inst)

    with TileContext(nc) as tc:
        with tc.tile_pool(name="sbuf", bufs=1) as pool:
            # Selected-row indices: cols 0-1 = global rows (halves 0/1) for
            # the gathers, cols 2-3 = half-local rows for the scatters.
            idx_t = pool.tile([128, 4], i32)
            nc.sync.dma_start(out=idx_t[:, 0:1], in_=idx[0:128, 0:1])
            nc.sync.dma_start(out=idx_t[:, 1:2], in_=idx[128:256, 0:1])
            nc.sync.dma_start(out=idx_t[:, 2:3], in_=idx[0:128, 1:2])
            nc.sync.dma_start(out=idx_t[:, 3:4], in_=idx[128:256, 1:2])

            # Bulk copy: out[:] = emb[:], straight DRAM->DRAM (most
            # engine-efficient path: each byte passes one SDMA engine once).
            # Split into chunks (finer descriptors -> all 16 SDMA engines
            # saturate early) and alternate the two HWDGE rings (SP + ACT).
            # Half-0: fine chunks for a fast ramp. Half-1 (whose completion
            # gates the tail scatter): fewer, larger chunks so the scatter
            # waits on fewer DMA-lane completion receipts.
            half_rows = ROWS // 2
            chunks_per_half = (n_copy_chunks // 2, 2)
            copy_halves = ([], [])
            k = 0
            for h in range(2):
                n_ch = chunks_per_half[h]
                assert half_rows % n_ch == 0
                rp = half_rows // n_ch
                for ch in range(n_ch):
                    dsl = slice(ch * rp, (ch + 1) * rp)
                    ssl = slice(h * half_rows + ch * rp, h * half_rows + (ch + 1) * rp)
                    eng = nc.sync if k % 2 == 0 else nc.scalar
                    k += 1
                    copy_halves[h].append(
                        eng.dma_start(out=outs[h][dsl, :], in_=emb[ssl, :])
                    )

            # Gather the 2x128 selected rows into one [128, 2, EMB] tile
            # (row h*128+p lands at partition p, slot h).
            rows3 = pool.tile([128, 2, EMB], f32)
            for half in range(2):
                hidx = idx_t[:, half : half + 1]
                rows = rows3[:, half, :]
                nc.gpsimd.indirect_dma_start(
                    out=rows,
                    out_offset=None,
                    in_=emb[:, :],
                    in_offset=bass.IndirectOffsetOnAxis(ap=hidx, axis=0),
                )
                # All compute on DVE (using ACT would pull in LoadActFuncSet
                # table loads that gate the ACT-ring copy chunks, and keeps
                # every op's cross-engine waits trivial): square + reduce for
                # sumsq, then rsqrt via the int32 bit-hack seed + 3 Newton
                # iterations (fp32-accurate), then scale = 1 + eps*rsqrt.
                sq = pool.tile([128, EMB], f32, tag=f"sq{half}")
                ss = pool.tile([128, 1], f32, tag=f"ss{half}")
                nc.vector.tensor_tensor(
                    out=sq[:], in0=rows, in1=rows, op=mybir.AluOpType.mult
                )
                nc.vector.reduce_sum(
                    out=ss[:], in_=sq[:], axis=mybir.AxisListType.X
                )
                # z0 = bitcast_f32(0x5f3759df - (bitcast_i32(ss) >> 1))
                zi = pool.tile([128, 1], i32, tag=f"zi{half}")
                nc.vector.tensor_scalar(
                    out=zi[:],
                    in0=ss[:].bitcast(i32),
                    scalar1=1,
                    scalar2=None,
                    op0=mybir.AluOpType.arith_shift_right,
                )
                # zi = ~(zi) + (0x5f3759df + 1)  ==  0x5f3759df - zi
                # (bitwise and arith ops can't share one TensorScalar)
                nc.vector.tensor_scalar(
                    out=zi[:],
                    in0=zi[:],
                    scalar1=-1,
                    scalar2=None,
                    op0=mybir.AluOpType.bitwise_xor,
                )
                nc.vector.tensor_scalar(
                    out=zi[:],
                    in0=zi[:],
                    scalar1=0x5F3759DF + 1,
                    scalar2=None,
                    op0=mybir.AluOpType.add,
                )
                z = zi[:].bitcast(f32)
                t0 = pool.tile([128, 1], f32, tag=f"t0{half}")
                for _ in range(3):
                    # z <- z * (1.5 - 0.5 * ss * z * z)
                    nc.vector.tensor_tensor(
                        out=t0[:], in0=z, in1=z, op=mybir.AluOpType.mult
                    )
                    nc.vector.tensor_tensor(
                        out=t0[:], in0=t0[:], in1=ss[:], op=mybir.AluOpType.mult
                    )
                    nc.vector.tensor_scalar(
                        out=t0[:],
                        in0=t0[:],
                        scalar1=-0.5,
                        scalar2=1.5,
                        op0=mybir.AluOpType.mult,
                        op1=mybir.AluOpType.add,
                    )
                    nc.vector.tensor_tensor(
                        out=zi[:].bitcast(f32), in0=z, in1=t0[:],
                        op=mybir.AluOpType.mult,
                    )
                # scale = 1 + EPSILON * rsqrt(sumsq)
                scl = pool.tile([128, 1], f32, tag=f"scl{half}")
                nc.vector.tensor_scalar(
                    out=scl[:],
                    in0=z,
                    scalar1=EPSILON,
                    scalar2=1.0,
                    op0=mybir.AluOpType.mult,
                    op1=mybir.AluOpType.add,
                )
                nc.vector.tensor_scalar_mul(out=rows, in0=rows, scalar1=scl[:, :1])

            # Each scatter lands after ITS half's copy chunks only (disjoint
            # output tensors -> no cross-half or scatter-scatter edges).
            for half in range(2):
                lidx = idx_t[:, 2 + half : 3 + half]
                si = nc.gpsimd.indirect_dma_start(
                    out=outs[half][:, :],
                    out_offset=bass.IndirectOffsetOnAxis(ap=lidx, axis=0),
                    in_=rows3[:, half, :],
                    in_offset=None,
                )
                for ci in copy_halves[half]:
                    add_dep_helper(raw(si), raw(ci), reason="scatter after half copy")

    # run_bass_via_pjrt binds the exec primitive without finalizing; Bacc
    # needs finalize() to run its compile() pipeline (register allocation,
    # event-semaphore legalization of multi-wait instructions, ...).
    nc.finalize()
    return nc


def _install_ntff_hook_shim():
    """This image lacks antenv.axon_hooks (which bass_utils imports for
    trace=True under axon); synthesize it from trn_boot's ctypes NTFF
    profiler. Profiling-only — never needed for plain execution."""
    import sys
    import types

    try:
        import antenv.axon_hooks  # noqa: F401

        return
    except ImportError:
        pass
    try:
        import antenv
        from trn_agent_boot.trn_boot import _ntff_profile_via_ctypes
    except ImportError:
        return
    hook = _ntff_profile_via_ctypes("/opt/axon/libaxon_pjrt.so")
    mod = types.ModuleType("antenv.axon_hooks")
    mod.get_axon_ntff_profile_hook = lambda: hook
    mod.set_axon_ntff_profile_hook = lambda h: None
    sys.modules["antenv.axon_hooks"] = mod
    antenv.axon_hooks = mod


LAST_EXEC_NS = None


def kernel(emb, data, dpadder):
    global LAST_EXEC_NS
    emb = np.ascontiguousarray(np.asarray(emb), dtype=np.float32)
    dpad = np.asarray(dpadder)
    L = _compute_L(dpad)

    from concourse.bass_utils import run_bass_kernel_spmd

    nc = _build_program()

    t = np.arange(SHARD_T, dtype=np.int64)
    in_maps = []
    for s in range(NCORES):
        Ls = L[s * SHARD_T : (s + 1) * SHARD_T].astype(np.int64)
        gidx = (t * BZ + Ls).astype(np.int32)
        lidx = gidx - np.where(t >= SHARD_T // 2, ROWS // 2, 0).astype(np.int32)
        rowidx = np.ascontiguousarray(np.stack([gidx, lidx], axis=1))
        shard = emb[s * SHARD_T : (s + 1) * SHARD_T].reshape(ROWS, EMB)
        in_maps.append({"emb": shard, "idx": rowidx})

    trace = os.environ.get("KERNEL_TRACE", "0") == "1"
    if trace:
        _install_ntff_hook_shim()
    res = run_bass_kernel_spmd(
        nc, in_maps, list(range(NCORES)), trace=trace
    )
    if trace:
        LAST_EXEC_NS = res.exec_time_ns
        print(f"HW exec time: {res.exec_time_ns} ns")
        print(f"mean exec time: {res.mean_exec_time_ns} ns")
        if res.instructions_and_trace is not None:
            print(f"trace: {res.instructions_and_trace[1]}")

    A = np.concatenate(
        [
            np.concatenate([r["out0"], r["out1"]], axis=0).reshape(
                SHARD_T, BZ, EMB
            )
            for r in res.results
        ],
        axis=0,
    )
    L_dtype = np.int64 if np.asarray(data).dtype == np.int64 else np.int32
    return A, L.astype(L_dtype)
